# revision 41
# baseline (speedup 1.0000x reference)
"""Multi-head causal attention (B=4, T=2048, C=1024, H=16, DH=64) on 8 trn2
NeuronCores.

Sharding: core = (batch, head-half): core 2*b+g computes heads g*8..g*8+8 of
batch b, including the partial output projection with the matching 512 rows
of Wp (tensor-parallel). Host-side unshard sums the two bf16 partials per
batch and adds bp.

Design (v2 ~240us, from ~289us baseline):
  - Q/K projections in fp8e4 DoubleRow (x at 16x, Wq/Wk at 64x, folded out
    via the copy scale and exp scale): same streaming rate as bf16 on this
    toolchain but half the instructions/LDWs. V/AV/outproj stay bf16 --
    fp8 there would put ~2.5% on the output values and fail the 2e-2 gate;
    the Q/K path is protected by the tiny 1/sqrt(C)=1/32 softmax scale.
  - QK row-tiled: each head's [64 x s] K slab sits at base partition 0/64;
    the two K=64 matmuls occupy disjoint PE row groups and dual-issue.
  - Causal mask: 0/1 tril multiply on DVE applied to the exp output's
    diagonal block (cheaper than the old PE mask-matmul accumulate).
  - ScalarE exp is the pacing engine late (~152us); emission order is the
    schedule. QK(si) is emitted BEFORE the trailing AVs (trail >= 2 slots,
    drained two at a time), so the exp feed never serializes behind the
    AV consume-wait -- this decoupling was worth ~20us.
  - Projection psum copy-outs ride ScalarE for windows 0-2 (ACT has slack
    there, DVE is the early bottleneck) and DVE for window 3's chains.
  - Window inserts: proj(tj+1) interleaves into window tj; ALL outproj for
    windows 0-2 is deferred into window 3 (ACT-bound there, PE has slack);
    outproj(3) + ScalarE casts form the tail.
  - Normalization: O^T/L with L from the V ones-column (psum row 64). o_ps
    frees via an immediate DVE copy (o_stage); the L DRAM-broadcast
    roundtrip + reciprocal run on DVE and the multiplies on GpSimd, all off
    the PE/ACT critical path.
  - Inputs arrive pre-tiled/packed from the host (x as [tj, p, c, 512]
    tiles bf16 + fp8, fp8 weights pre-packed for DoubleRow's [Ki,2,dim]
    AP) so every DMA is a single transfer; output is stored bf16.
"""
import numpy as np
import ml_dtypes

import concourse.mybir as mybir
import concourse.tile as tile
from concourse import bacc, bass_utils

F32 = mybir.dt.float32
BF16 = mybir.dt.bfloat16
FP8 = mybir.dt.float8e4
XS = 16.0             # fp8 scale for x
WS = 64.0             # fp8 scale for Wq/Wk
QKS = 1.0 / 64.0      # psum -> q_sb/k_sb copy scale (leaves q,k at 16x)

B, T, C, H, DH = 4, 2048, 1024, 16, 64
HG = H // 2          # heads per core (8)
CC = C // 128        # contraction chunks (8)
TJ = 512             # query chunk width
NTJ = T // TJ        # 4
SCALE = 1.0 / 32.0 / (16.0 * 16.0)   # 1/sqrt(C), q/k stored at 16x

TRACE = False
_NC_CACHE = {}


def _build():
    nc = bacc.Bacc(trn_type="TRN2", target_bir_lowering=False, debug=False)

    xtt = nc.dram_tensor("xtt", [NTJ, 128, CC, TJ], BF16,
                         kind="ExternalInput")
    xtt8 = nc.dram_tensor("xtt8", [NTJ, 128, CC, TJ], FP8,
                          kind="ExternalInput")
    # packed fp8 DoubleRow Q/K weights: [p, qk, mg, cpair, j, dhcol]
    w8 = nc.dram_tensor("w8", [128, 2, 4, 4, 2, 128], FP8,
                        kind="ExternalInput")
    wr = nc.dram_tensor("wr", [128, 4 * C], BF16, kind="ExternalInput")
    wp = nc.dram_tensor("wp", [HG * DH, C], BF16, kind="ExternalInput")
    trilm = nc.dram_tensor("trilm", [128, 2, 128], BF16, kind="ExternalInput")
    ones8 = nc.dram_tensor("ones8", [128, 64], BF16, kind="ExternalInput")
    out = nc.dram_tensor("out", [T, C], BF16, kind="ExternalOutput")

    l_dram = nc.dram_tensor("l_scratch", [HG * NTJ, TJ], F32)

    with tile.TileContext(nc) as tc:
        with (
            tc.tile_pool(name="persist", bufs=1) as persist,
            tc.tile_pool(name="ppool", bufs=6) as ppool,
            tc.tile_pool(name="npool", bufs=4) as npool,
            tc.tile_pool(name="outpool", bufs=4) as outpool,
            tc.tile_pool(name="pp", bufs=2, space="PSUM") as pp,
            tc.tile_pool(name="aps", bufs=2, space="PSUM") as aps,
        ):
            trilm_sb = persist.tile([128, 2, 128], BF16)
            nc.sync.dma_start(out=trilm_sb, in_=trilm.ap())
            ones_sb = persist.tile([128, 64], BF16)
            nc.sync.dma_start(out=ones_sb, in_=ones8.ap())

            # resident activations/weights, all bf16
            k_sb = [persist.tile([128, T], BF16, name=f"k_{mg}")
                    for mg in range(HG // 2)]
            q_sb = [persist.tile([128, T], BF16, name=f"q_{mg}")
                    for mg in range(HG // 2)]
            o_sb = [persist.tile([128, T], BF16, name=f"o_{mg}")
                    for mg in range(HG // 2)]
            v_sb = [persist.tile([128, HG, DH + 1], BF16, name=f"v_{si}")
                    for si in range(T // 128)]
            # bf16 weights: V columns only, 512 per c-chunk
            w_all = persist.tile([128, 4 * C], BF16)
            w8_sb = persist.tile([128, 2, 4, 4, 2, 128], FP8)
            xt_all = persist.tile([128, CC, T], BF16)
            xt8_all = persist.tile([128, CC, T], FP8)
            xt_sb = [xt_all[:, c, :] for c in range(CC)]

            def w_v(c):
                return w_all[:, c * 512:(c + 1) * 512]
            wp_sb = [persist.tile([128, C], BF16, name=f"wp_{hp}")
                     for hp in range(HG // 2)]

            def dma_x(tn):
                tsl = slice(tn * TJ, (tn + 1) * TJ)
                nc.sync.dma_start(out=xt_all[:, :, tsl],
                                  in_=xtt.ap()[tn])

            def dma_x8(tn):
                tsl = slice(tn * TJ, (tn + 1) * TJ)
                nc.sync.dma_start(out=xt8_all[:, :, tsl],
                                  in_=xtt8.ap()[tn])

            # first-needed first: fp8 q/k mg0 weights + x8(0) (q/k proj for
            # unit 0), then bf16 x(0) + v weights, then the rest.
            # First three loads ride three different engines' DGEs so their
            # ~0.6-1us trigger configs overlap instead of serializing on SP.
            # wp last: outproj doesn't run until window 3.
            nc.scalar.dma_start(out=w8_sb[:, :, 0:1], in_=w8.ap()[:, :, 0:1])
            dma_x8(0)
            dma_x(0)
            nc.sync.dma_start(out=w_all, in_=wr.ap())
            nc.sync.dma_start(out=w8_sb[:, :, 1:4], in_=w8.ap()[:, :, 1:4])
            dma_x8(1)
            dma_x(1)
            for hp in range(HG // 2):
                nc.sync.dma_start(out=wp_sb[hp],
                                  in_=wp.ap()[hp * 128:(hp + 1) * 128, :])

            o_ps = [aps.tile([DH + 1, TJ], F32, name=f"o_ps{i}", bufs=1)
                    for i in range(2)]



            # ---- emission thunks -------------------------------------
            proj_state = {}

            def proj_half(tn, kind, mg, half):
                """One half of a projection psum chain.
                kind: 0=Q, 1=K (fp8 DoubleRow, 2 c-pairs per half),
                2=V (bf16, 4 c-chunks per half). half 1 copies out."""
                tsl = slice(tn * TJ, (tn + 1) * TJ)
                cs = range(4) if half == 0 else range(4, CC)
                key = (tn, kind, mg)
                if half == 0:
                    t_ = pp.tile([128, TJ], F32, name="prj", tag="pp")
                    proj_state[key] = t_
                else:
                    t_ = proj_state.pop(key)
                # psum->sbuf copy engine: ScalarE for windows 0-2's chains
                # (ACT has slack there; DVE is the W0/W1 bottleneck), DVE
                # for window 3's (emitted in W2 where ACT is ~90% busy).
                use_act = tn < 3

                def ceng(dst, s_):
                    if use_act:
                        nc.scalar.copy(dst, s_)
                    else:
                        nc.vector.tensor_copy(dst, s_)
                if kind in (0, 1):
                    cps = range(2) if half == 0 else range(2, 4)
                    for cp in cps:
                        nc.tensor.matmul(
                            t_, w8_sb[:, kind, mg, cp],
                            xt8_all[:, 2 * cp:2 * cp + 2, tsl],
                            start=(cp == 0), stop=(cp == 3),
                            perf_mode=mybir.MatmulPerfMode.DoubleRow)
                    if half == 1:
                        dst = (q_sb if kind == 0 else k_sb)[mg][:, tsl]
                        if use_act:
                            nc.scalar.mul(dst, t_, QKS)
                        else:
                            nc.vector.tensor_scalar_mul(dst, t_, QKS)
                else:
                    si = 4 * tn + mg
                    ssl = slice(si * 128, (si + 1) * 128)
                    for c in cs:
                        nc.tensor.matmul(
                            t_, xt_sb[c][:, ssl], w_v(c),
                            start=(c == 0), stop=(c == CC - 1))
                    if half == 1:
                        ceng(
                            v_sb[si][:, :, 0:DH],
                            t_.rearrange("p (h d) -> p h d", h=HG))
                        nc.vector.tensor_copy(
                            out=v_sb[si][:, :, DH:DH + 1],
                            in_=ones_sb[:, 0:HG, None])

            def proj_thunks(tn, jobs=None):
                if jobs is None:
                    jobs = [(k, m) for m in range(4) for k in (0, 1, 2)]
                th = []
                for kind, mg in jobs:
                    for half in (0, 1):
                        th.append(lambda k=kind, m=mg, h=half:
                                  proj_half(tn, k, m, h))
                return th

            def outproj_tile(tj, k4, en):
                ti = 4 * tj + k4
                tsl = slice(ti * 128, (ti + 1) * 128)
                esl = slice(en * TJ, (en + 1) * TJ)
                op_ps = pp.tile([128, TJ], F32, name="op", tag="pp")
                for hp in range(HG // 2):
                    nc.tensor.matmul(
                        op_ps, o_sb[hp][:, tsl], wp_sb[hp][:, esl],
                        start=(hp == 0), stop=(hp == HG // 2 - 1))
                ob = outpool.tile([128, TJ], BF16, name="ob")
                if tj == NTJ - 1:
                    # tail: ScalarE is idle after the last exp
                    nc.scalar.copy(ob, op_ps)
                else:
                    nc.vector.tensor_copy(ob, op_ps)
                nc.sync.dma_start(out=out.ap()[tsl, esl], in_=ob)

            def outproj_thunks(tj):
                return [lambda k=k4, e=en: outproj_tile(tj, k, e)
                        for k4 in range(4) for en in range(2)]

            def attn_qk(hp, tj, si):
                r = si - 4 * tj
                toff = 0 if r < 0 else 128 * r
                ssl = slice(si * 128, (si + 1) * 128)
                qsl = slice(tj * TJ + toff, (tj + 1) * TJ)

                s_ps = aps.tile([128, 2, TJ], F32, name="s_ps", tag="s",
                                bufs=2)
                nc.tensor.matmul(
                    s_ps[:, 0, toff:TJ],
                    k_sb[hp][0:64, ssl], q_sb[hp][0:64, qsl],
                    start=True, stop=True)
                nc.tensor.matmul(
                    s_ps[:, 1, toff:TJ],
                    k_sb[hp][64:128, ssl], q_sb[hp][64:128, qsl],
                    start=True, stop=True)
                p_sb = ppool.tile([128, 2, TJ], BF16, name="p_sb", tag="p")
                nc.scalar.activation(
                    p_sb[:, :, toff:TJ], s_ps[:, :, toff:TJ],
                    mybir.ActivationFunctionType.Exp, scale=SCALE)
                if r >= 0:
                    # diag block: zero the above-diagonal entries of p with a
                    # 0/1 tril multiply on DVE (cheaper than PE mask matmuls)
                    nc.vector.tensor_tensor(
                        out=p_sb[:, :, toff:toff + 128],
                        in0=p_sb[:, :, toff:toff + 128],
                        in1=trilm_sb,
                        op=mybir.AluOpType.mult)
                return p_sb, toff

            def attn_av(hp, tj, si, nsi, p_sb, toff):
                h0, h1 = 2 * hp, 2 * hp + 1
                nc.tensor.matmul(
                    o_ps[0][:, toff:TJ], v_sb[si][:, h0, :],
                    p_sb[:, 0, toff:TJ],
                    start=(si == 0), stop=(si == nsi - 1))
                nc.tensor.matmul(
                    o_ps[1][:, toff:TJ], v_sb[si][:, h1, :],
                    p_sb[:, 1, toff:TJ],
                    start=(si == 0), stop=(si == nsi - 1))

            def attn_norm_last(hp, tj):
                # final unit: nothing reuses o_ps afterwards, so normalize
                # straight out of PSUM with L broadcast by a K=1 matmul
                # (ones stationary) instead of the DRAM roundtrip.
                tsl = slice(tj * TJ, (tj + 1) * TJ)
                for idx in range(2):
                    l_bf = npool.tile([DH + 1, TJ], BF16, name="l_bf")
                    nc.vector.tensor_copy(
                        l_bf[DH:DH + 1, :], o_ps[idx][DH:DH + 1, :])
                    lb_ps = pp.tile([128, TJ], F32, name="lbps", tag="pp")
                    nc.tensor.matmul(
                        lb_ps[0:64, :], ones_sb[DH:DH + 1, :],
                        l_bf[DH:DH + 1, :], start=True, stop=True)
                    linv = npool.tile([64, TJ], F32, name="linv")
                    nc.vector.reciprocal_approx_fast(linv, lb_ps[0:64, :])
                    if idx == 0:
                        nc.vector.tensor_tensor(
                            out=o_sb[hp][0:64, tsl], in0=o_ps[0][0:DH, :],
                            in1=linv, op=mybir.AluOpType.mult)
                    else:
                        o_tmp = npool.tile([64, TJ], BF16, name="o_tmp")
                        nc.vector.tensor_tensor(
                            out=o_tmp, in0=o_ps[1][0:DH, :],
                            in1=linv, op=mybir.AluOpType.mult)
                        nc.sync.dma_start(
                            out=o_sb[hp][64:128, tsl], in_=o_tmp)

            def attn_norm(hp, tj):
                if hp == HG // 2 - 1 and tj == NTJ - 1:
                    return attn_norm_last(hp, tj)
                # free o_ps fast (DVE copy), then normalize off-PE:
                # L row 64 -> DRAM -> broadcast to 64 partitions,
                # reciprocal, multiply; h1 lands via SBUF->SBUF DMA.
                tsl = slice(tj * TJ, (tj + 1) * TJ)
                for idx in range(2):
                    lrow = (hp * 2 + idx) * NTJ + tj
                    o_stage = npool.tile([DH + 1, TJ], F32, name="o_stage")
                    nc.vector.tensor_copy(o_stage, o_ps[idx])
                    nc.sync.dma_start(
                        out=l_dram.ap()[lrow:lrow + 1, :],
                        in_=o_stage[DH:DH + 1, :])
                    lb = npool.tile([64, TJ], F32, name="lb")
                    nc.sync.dma_start(
                        out=lb,
                        in_=l_dram.ap()[lrow:lrow + 1, :]
                        .to_broadcast((64, TJ)))
                    linv = npool.tile([64, TJ], F32, name="linv")
                    nc.vector.reciprocal_approx_fast(linv, lb)
                    if idx == 0:
                        nc.gpsimd.tensor_tensor(
                            out=o_sb[hp][0:64, tsl], in0=o_stage[0:DH, :],
                            in1=linv, op=mybir.AluOpType.mult)
                    else:
                        o_tmp = npool.tile([64, TJ], BF16, name="o_tmp")
                        nc.gpsimd.tensor_tensor(
                            out=o_tmp, in0=o_stage[0:DH, :],
                            in1=linv, op=mybir.AluOpType.mult)
                        nc.sync.dma_start(
                            out=o_sb[hp][64:128, tsl], in_=o_tmp)

            # ---- interleaved emission --------------------------------
            # window 0 starts after only Q0/K0/V0; the rest of proj(0)
            # interleaves into window 0 ahead of proj(1), ordered so each
            # unit's Q/K land before that unit's first chunk.
            # Slot emission order is QK(si) first, AV trailing by 2 slots:
            # the PE never waits on EXP(si-1) before issuing QK(si), so the
            # exp stream decouples from the AV consume wait. Outproj is
            # deferred to window 3 (ACT-bound there: PE has ~30us of slack);
            # windows 0-2 carry only the projection chains they must.
            for th in proj_thunks(0, [(0, 0), (1, 0), (2, 0)]):
                th()
            rest0 = proj_thunks(0, [(2, 1), (2, 2), (2, 3), (0, 1), (1, 1),
                                    (0, 2), (1, 2), (0, 3), (1, 3)])
            from collections import deque
            for tj in range(NTJ):
                # x for slice tj+1 must land before the interleaved
                # proj(tj+1) chains read it (slices 0,1 pre-issued)
                if 2 <= tj + 1 < NTJ:
                    dma_x8(tj + 1)
                    dma_x(tj + 1)
                inserts = []
                if tj == 0:
                    inserts += rest0
                if tj + 1 < NTJ:
                    inserts += proj_thunks(tj + 1)
                if tj == NTJ - 1:
                    for t_ in range(NTJ - 1):
                        inserts += outproj_thunks(t_)
                nsi = 4 * tj + 4
                slots = []  # (kind, args) in emission order
                for hp in range(HG // 2):
                    for si in range(nsi):
                        slots.append(("c", hp, si))
                    slots.append(("n", hp, None))
                n_slots = len(slots)
                n_ins = len(inserts)
                acc = 0.0
                ii = 0
                pend = {}
                for j, s in enumerate(slots):
                    hp = s[1]
                    pq = pend.setdefault(hp, deque())
                    if s[0] == "c":
                        si = s[2]
                        pq.append((si, attn_qk(hp, tj, si)))
                        # QK leads; AVs drain two at a time at trail >= 2
                        if len(pq) >= 4:
                            for _ in range(2):
                                psi, pa = pq.popleft()
                                attn_av(hp, tj, psi, nsi, *pa)
                    else:
                        while pq:
                            psi, pa = pq.popleft()
                            attn_av(hp, tj, psi, nsi, *pa)
                        attn_norm(hp, tj)
                    acc += n_ins / n_slots
                    # fire inserts only at slot-pair boundaries so the
                    # [QK,QK][AV x4] groups stay adjacent on the PE queue
                    if j % 2 == 1 or s[0] == "n":
                        while ii < n_ins and acc >= 1.0:
                            inserts[ii]()
                            ii += 1
                            acc -= 1.0
                while ii < n_ins:
                    inserts[ii]()
                    ii += 1
            for th in outproj_thunks(NTJ - 1):
                th()

    nc.compile()
    return nc


def _get_nc():
    if "nc" not in _NC_CACHE:
        _NC_CACHE["nc"] = _build()
    return _NC_CACHE["nc"]


def _make_in_maps(x, Wq, Wk, Wv, Wp):
    bf = ml_dtypes.bfloat16
    tril_h = np.where(
        np.arange(128)[:, None] > np.arange(128)[None, :],
        np.float32(0.0), np.float32(1.0)).astype(np.float32)
    trilm_h = np.ascontiguousarray(
        np.broadcast_to(tril_h[:, None, :], (128, 2, 128))).astype(bf)
    f8 = ml_dtypes.float8_e4m3
    in_maps = []
    for core in range(8):
        b, g = core // 2, core % 2
        heads = range(g * HG, (g + 1) * HG)
        wq = np.concatenate([Wq[h] for h in heads], axis=1)
        wk = np.concatenate([Wk[h] for h in heads], axis=1)
        wv = np.concatenate([Wv[h] for h in heads], axis=1)
        xt_f = x[b].T.astype(np.float32)  # [C, T]
        xtt_h = np.ascontiguousarray(
            xt_f.reshape(CC, 128, NTJ, TJ).transpose(2, 1, 0, 3)).astype(bf)
        xtt8_h = np.ascontiguousarray(
            (xt_f * XS).reshape(CC, 128, NTJ, TJ)
            .transpose(2, 1, 0, 3)).astype(f8)
        # w8[p, qk, mg, cp, j, m] = WS * w[(2cp+j)*128+p, mg*128+m]
        w8_h = np.empty((128, 2, 4, 4, 2, 128), np.float32)
        for qk, w_ in ((0, wq), (1, wk)):
            wr4 = (w_ * WS).reshape(4, 2, 128, 4, 128)  # [cp, j, p, mg, m]
            w8_h[:, qk] = wr4.transpose(2, 3, 0, 1, 4)  # [p, mg, cp, j, m]
        wr_h = np.empty((128, 4 * C), np.float32)
        for c in range(CC):
            wr_h[:, c * 512:(c + 1) * 512] = wv[c * 128:(c + 1) * 128]
        in_maps.append({
            "xtt": xtt_h,
            "xtt8": xtt8_h,
            "w8": w8_h.astype(f8),
            "wr": wr_h.astype(bf),
            "wp": np.ascontiguousarray(
                Wp[g * HG * DH:(g + 1) * HG * DH, :]).astype(bf),
            "trilm": trilm_h,
            "ones8": np.ones((128, 64), bf),
        })
    return in_maps


_LAST_RESULTS = {}


def kernel(x, Wq, Wk, Wv, Wp, bp):
    x = np.asarray(x, np.float32)
    Wq = np.asarray(Wq, np.float32)
    Wk = np.asarray(Wk, np.float32)
    Wv = np.asarray(Wv, np.float32)
    Wp = np.asarray(Wp, np.float32)
    bp = np.asarray(bp, np.float32)

    nc = _get_nc()
    in_maps = _make_in_maps(x, Wq, Wk, Wv, Wp)
    res = bass_utils.run_bass_kernel_spmd(
        nc, in_maps, core_ids=list(range(8)), trace=TRACE)
    _LAST_RESULTS["res"] = res

    out = np.empty((B, T, C), np.float32)
    for b in range(B):
        out[b] = (res.results[2 * b]["out"].astype(np.float32)
                  + res.results[2 * b + 1]["out"].astype(np.float32) + bp)
    return out



# revision 42
# speedup vs baseline: 1.0013x; 1.0013x over previous
"""Multi-head causal attention (B=4, T=2048, C=1024, H=16, DH=64) on 8 trn2
NeuronCores.

Sharding: core = (batch, head-half): core 2*b+g computes heads g*8..g*8+8 of
batch b, including the partial output projection with the matching 512 rows
of Wp (tensor-parallel). Host-side unshard sums the two bf16 partials per
batch and adds bp.

Design (v2 ~240us, from ~289us baseline):
  - Q/K projections in fp8e4 DoubleRow (x at 16x, Wq/Wk at 64x, folded out
    via the copy scale and exp scale): same streaming rate as bf16 on this
    toolchain but half the instructions/LDWs. V/AV/outproj stay bf16 --
    fp8 there would put ~2.5% on the output values and fail the 2e-2 gate;
    the Q/K path is protected by the tiny 1/sqrt(C)=1/32 softmax scale.
  - QK row-tiled: each head's [64 x s] K slab sits at base partition 0/64;
    the two K=64 matmuls occupy disjoint PE row groups and dual-issue.
  - Causal mask: 0/1 tril multiply on DVE applied to the exp output's
    diagonal block (cheaper than the old PE mask-matmul accumulate).
  - ScalarE exp is the pacing engine late (~152us); emission order is the
    schedule. QK(si) is emitted BEFORE the trailing AVs (trail >= 2 slots,
    drained two at a time), so the exp feed never serializes behind the
    AV consume-wait -- this decoupling was worth ~20us.
  - Projection psum copy-outs ride ScalarE for windows 0-2 (ACT has slack
    there, DVE is the early bottleneck) and DVE for window 3's chains.
  - Window inserts: proj(tj+1) interleaves into window tj; ALL outproj for
    windows 0-2 is deferred into window 3 (ACT-bound there, PE has slack);
    outproj(3) + ScalarE casts form the tail.
  - Normalization: O^T/L with L from the V ones-column (psum row 64). o_ps
    frees via an immediate DVE copy (o_stage); the L DRAM-broadcast
    roundtrip + reciprocal run on DVE and the multiplies on GpSimd, all off
    the PE/ACT critical path.
  - Inputs arrive pre-tiled/packed from the host (x as [tj, p, c, 512]
    tiles bf16 + fp8, fp8 weights pre-packed for DoubleRow's [Ki,2,dim]
    AP) so every DMA is a single transfer; output is stored bf16.
"""
import numpy as np
import ml_dtypes

import concourse.mybir as mybir
import concourse.tile as tile
from concourse import bacc, bass_utils

F32 = mybir.dt.float32
BF16 = mybir.dt.bfloat16
FP8 = mybir.dt.float8e4
XS = 16.0             # fp8 scale for x
WS = 64.0             # fp8 scale for Wq/Wk
QKS = 1.0 / 64.0      # psum -> q_sb/k_sb copy scale (leaves q,k at 16x)

B, T, C, H, DH = 4, 2048, 1024, 16, 64
HG = H // 2          # heads per core (8)
CC = C // 128        # contraction chunks (8)
TJ = 512             # query chunk width
NTJ = T // TJ        # 4
SCALE = 1.0 / 32.0 / (16.0 * 16.0)   # 1/sqrt(C), q/k stored at 16x

TRACE = False
_NC_CACHE = {}


def _build():
    nc = bacc.Bacc(trn_type="TRN2", target_bir_lowering=False, debug=False)

    xtt = nc.dram_tensor("xtt", [NTJ, 128, CC, TJ], BF16,
                         kind="ExternalInput")
    xtt8 = nc.dram_tensor("xtt8", [NTJ, 128, CC, TJ], FP8,
                          kind="ExternalInput")
    # packed fp8 DoubleRow Q/K weights: [p, qk, mg, cpair, j, dhcol]
    w8 = nc.dram_tensor("w8", [128, 2, 4, 4, 2, 128], FP8,
                        kind="ExternalInput")
    wr = nc.dram_tensor("wr", [128, 4 * C], BF16, kind="ExternalInput")
    wp = nc.dram_tensor("wp", [HG * DH, C], BF16, kind="ExternalInput")
    trilm = nc.dram_tensor("trilm", [128, 2, 128], BF16, kind="ExternalInput")
    ones8 = nc.dram_tensor("ones8", [128, 64], BF16, kind="ExternalInput")
    out = nc.dram_tensor("out", [T, C], BF16, kind="ExternalOutput")

    l_dram = nc.dram_tensor("l_scratch", [HG * NTJ, TJ], F32)

    with tile.TileContext(nc) as tc:
        with (
            tc.tile_pool(name="persist", bufs=1) as persist,
            tc.tile_pool(name="ppool", bufs=6) as ppool,
            tc.tile_pool(name="npool", bufs=4) as npool,
            tc.tile_pool(name="outpool", bufs=4) as outpool,
            tc.tile_pool(name="pp", bufs=2, space="PSUM") as pp,
            tc.tile_pool(name="aps", bufs=2, space="PSUM") as aps,
        ):
            trilm_sb = persist.tile([128, 2, 128], BF16)
            nc.sync.dma_start(out=trilm_sb, in_=trilm.ap())
            ones_sb = persist.tile([128, 64], BF16)
            nc.sync.dma_start(out=ones_sb, in_=ones8.ap())

            # resident activations/weights, all bf16
            k_sb = [persist.tile([128, T], BF16, name=f"k_{mg}")
                    for mg in range(HG // 2)]
            q_sb = [persist.tile([128, T], BF16, name=f"q_{mg}")
                    for mg in range(HG // 2)]
            o_sb = [persist.tile([128, T], BF16, name=f"o_{mg}")
                    for mg in range(HG // 2)]
            v_sb = [persist.tile([128, HG, DH + 1], BF16, name=f"v_{si}")
                    for si in range(T // 128)]
            # bf16 weights: V columns only, 512 per c-chunk
            w_all = persist.tile([128, 4 * C], BF16)
            w8_sb = persist.tile([128, 2, 4, 4, 2, 128], FP8)
            xt_all = persist.tile([128, CC, T], BF16)
            xt8_all = persist.tile([128, CC, T], FP8)
            xt_sb = [xt_all[:, c, :] for c in range(CC)]

            def w_v(c):
                return w_all[:, c * 512:(c + 1) * 512]
            wp_sb = [persist.tile([128, C], BF16, name=f"wp_{hp}")
                     for hp in range(HG // 2)]

            def dma_x(tn):
                tsl = slice(tn * TJ, (tn + 1) * TJ)
                nc.sync.dma_start(out=xt_all[:, :, tsl],
                                  in_=xtt.ap()[tn])

            def dma_x8(tn):
                tsl = slice(tn * TJ, (tn + 1) * TJ)
                nc.sync.dma_start(out=xt8_all[:, :, tsl],
                                  in_=xtt8.ap()[tn])

            # first-needed first: fp8 q/k mg0 weights + x8(0) (q/k proj for
            # unit 0), then bf16 x(0) + v weights, then the rest.
            # First three loads ride three different engines' DGEs so their
            # ~0.6-1us trigger configs overlap instead of serializing on SP.
            # wp last: outproj doesn't run until window 3.
            nc.scalar.dma_start(out=w8_sb[:, :, 0:1], in_=w8.ap()[:, :, 0:1])
            dma_x8(0)
            dma_x(0)
            nc.sync.dma_start(out=w_all, in_=wr.ap())
            nc.sync.dma_start(out=w8_sb[:, :, 1:4], in_=w8.ap()[:, :, 1:4])
            dma_x8(1)
            dma_x(1)
            for hp in range(HG // 2):
                nc.sync.dma_start(out=wp_sb[hp],
                                  in_=wp.ap()[hp * 128:(hp + 1) * 128, :])

            o_ps = [aps.tile([DH + 1, TJ], F32, name=f"o_ps{i}", bufs=1)
                    for i in range(2)]



            # ---- emission thunks -------------------------------------
            proj_state = {}

            def proj_half(tn, kind, mg, half):
                """One half of a projection psum chain.
                kind: 0=Q, 1=K (fp8 DoubleRow, 2 c-pairs per half),
                2=V (bf16, 4 c-chunks per half). half 1 copies out."""
                tsl = slice(tn * TJ, (tn + 1) * TJ)
                cs = range(4) if half == 0 else range(4, CC)
                key = (tn, kind, mg)
                if half == 0:
                    t_ = pp.tile([128, TJ], F32, name="prj", tag="pp")
                    proj_state[key] = t_
                else:
                    t_ = proj_state.pop(key)
                # psum->sbuf copy engine: ScalarE for windows 0-2's q/k
                # chains (ACT has slack there; DVE is the W0/W1 bottleneck),
                # DVE for V and for window 3's chains (ACT ~90% busy in W2).
                use_act = tn < 3 and kind != 2

                def ceng(dst, s_):
                    if use_act:
                        nc.scalar.copy(dst, s_)
                    else:
                        nc.vector.tensor_copy(dst, s_)
                if kind in (0, 1):
                    cps = range(2) if half == 0 else range(2, 4)
                    for cp in cps:
                        nc.tensor.matmul(
                            t_, w8_sb[:, kind, mg, cp],
                            xt8_all[:, 2 * cp:2 * cp + 2, tsl],
                            start=(cp == 0), stop=(cp == 3),
                            perf_mode=mybir.MatmulPerfMode.DoubleRow)
                    if half == 1:
                        dst = (q_sb if kind == 0 else k_sb)[mg][:, tsl]
                        if use_act:
                            nc.scalar.mul(dst, t_, QKS)
                        else:
                            nc.vector.tensor_scalar_mul(dst, t_, QKS)
                else:
                    si = 4 * tn + mg
                    ssl = slice(si * 128, (si + 1) * 128)
                    for c in cs:
                        nc.tensor.matmul(
                            t_, xt_sb[c][:, ssl], w_v(c),
                            start=(c == 0), stop=(c == CC - 1))
                    if half == 1:
                        ceng(
                            v_sb[si][:, :, 0:DH],
                            t_.rearrange("p (h d) -> p h d", h=HG))
                        nc.vector.tensor_copy(
                            out=v_sb[si][:, :, DH:DH + 1],
                            in_=ones_sb[:, 0:HG, None])

            def proj_thunks(tn, jobs=None):
                if jobs is None:
                    jobs = [(k, m) for m in range(4) for k in (0, 1, 2)]
                th = []
                for kind, mg in jobs:
                    for half in (0, 1):
                        th.append(lambda k=kind, m=mg, h=half:
                                  proj_half(tn, k, m, h))
                return th

            def outproj_tile(tj, k4, en):
                ti = 4 * tj + k4
                tsl = slice(ti * 128, (ti + 1) * 128)
                esl = slice(en * TJ, (en + 1) * TJ)
                op_ps = pp.tile([128, TJ], F32, name="op", tag="pp")
                for hp in range(HG // 2):
                    nc.tensor.matmul(
                        op_ps, o_sb[hp][:, tsl], wp_sb[hp][:, esl],
                        start=(hp == 0), stop=(hp == HG // 2 - 1))
                ob = outpool.tile([128, TJ], BF16, name="ob")
                if tj == NTJ - 1:
                    # tail: ScalarE is idle after the last exp
                    nc.scalar.copy(ob, op_ps)
                else:
                    nc.vector.tensor_copy(ob, op_ps)
                nc.sync.dma_start(out=out.ap()[tsl, esl], in_=ob)

            def outproj_thunks(tj):
                return [lambda k=k4, e=en: outproj_tile(tj, k, e)
                        for k4 in range(4) for en in range(2)]

            def attn_qk(hp, tj, si):
                r = si - 4 * tj
                toff = 0 if r < 0 else 128 * r
                ssl = slice(si * 128, (si + 1) * 128)
                qsl = slice(tj * TJ + toff, (tj + 1) * TJ)

                s_ps = aps.tile([128, 2, TJ], F32, name="s_ps", tag="s",
                                bufs=2)
                nc.tensor.matmul(
                    s_ps[:, 0, toff:TJ],
                    k_sb[hp][0:64, ssl], q_sb[hp][0:64, qsl],
                    start=True, stop=True)
                nc.tensor.matmul(
                    s_ps[:, 1, toff:TJ],
                    k_sb[hp][64:128, ssl], q_sb[hp][64:128, qsl],
                    start=True, stop=True)
                p_sb = ppool.tile([128, 2, TJ], BF16, name="p_sb", tag="p")
                nc.scalar.activation(
                    p_sb[:, :, toff:TJ], s_ps[:, :, toff:TJ],
                    mybir.ActivationFunctionType.Exp, scale=SCALE)
                if r >= 0:
                    # diag block: zero the above-diagonal entries of p with a
                    # 0/1 tril multiply on DVE (cheaper than PE mask matmuls)
                    nc.vector.tensor_tensor(
                        out=p_sb[:, :, toff:toff + 128],
                        in0=p_sb[:, :, toff:toff + 128],
                        in1=trilm_sb,
                        op=mybir.AluOpType.mult)
                return p_sb, toff

            def attn_av(hp, tj, si, nsi, p_sb, toff):
                h0, h1 = 2 * hp, 2 * hp + 1
                nc.tensor.matmul(
                    o_ps[0][:, toff:TJ], v_sb[si][:, h0, :],
                    p_sb[:, 0, toff:TJ],
                    start=(si == 0), stop=(si == nsi - 1))
                nc.tensor.matmul(
                    o_ps[1][:, toff:TJ], v_sb[si][:, h1, :],
                    p_sb[:, 1, toff:TJ],
                    start=(si == 0), stop=(si == nsi - 1))

            def attn_norm_last(hp, tj):
                # final unit: nothing reuses o_ps afterwards, so normalize
                # straight out of PSUM with L broadcast by a K=1 matmul
                # (ones stationary) instead of the DRAM roundtrip.
                tsl = slice(tj * TJ, (tj + 1) * TJ)
                for idx in range(2):
                    l_bf = npool.tile([DH + 1, TJ], BF16, name="l_bf")
                    nc.vector.tensor_copy(
                        l_bf[DH:DH + 1, :], o_ps[idx][DH:DH + 1, :])
                    lb_ps = pp.tile([128, TJ], F32, name="lbps", tag="pp")
                    nc.tensor.matmul(
                        lb_ps[0:64, :], ones_sb[DH:DH + 1, :],
                        l_bf[DH:DH + 1, :], start=True, stop=True)
                    linv = npool.tile([64, TJ], F32, name="linv")
                    nc.vector.reciprocal_approx_fast(linv, lb_ps[0:64, :])
                    if idx == 0:
                        nc.vector.tensor_tensor(
                            out=o_sb[hp][0:64, tsl], in0=o_ps[0][0:DH, :],
                            in1=linv, op=mybir.AluOpType.mult)
                    else:
                        o_tmp = npool.tile([64, TJ], BF16, name="o_tmp")
                        nc.vector.tensor_tensor(
                            out=o_tmp, in0=o_ps[1][0:DH, :],
                            in1=linv, op=mybir.AluOpType.mult)
                        nc.sync.dma_start(
                            out=o_sb[hp][64:128, tsl], in_=o_tmp)

            def attn_norm(hp, tj):
                if hp == HG // 2 - 1 and tj == NTJ - 1:
                    return attn_norm_last(hp, tj)
                # free o_ps fast (DVE copy), then normalize off-PE:
                # L row 64 -> DRAM -> broadcast to 64 partitions,
                # reciprocal, multiply; h1 lands via SBUF->SBUF DMA.
                tsl = slice(tj * TJ, (tj + 1) * TJ)
                for idx in range(2):
                    lrow = (hp * 2 + idx) * NTJ + tj
                    o_stage = npool.tile([DH + 1, TJ], F32, name="o_stage")
                    nc.vector.tensor_copy(o_stage, o_ps[idx])
                    nc.sync.dma_start(
                        out=l_dram.ap()[lrow:lrow + 1, :],
                        in_=o_stage[DH:DH + 1, :])
                    lb = npool.tile([64, TJ], F32, name="lb")
                    nc.sync.dma_start(
                        out=lb,
                        in_=l_dram.ap()[lrow:lrow + 1, :]
                        .to_broadcast((64, TJ)))
                    linv = npool.tile([64, TJ], F32, name="linv")
                    nc.vector.reciprocal_approx_fast(linv, lb)
                    if idx == 0:
                        nc.gpsimd.tensor_tensor(
                            out=o_sb[hp][0:64, tsl], in0=o_stage[0:DH, :],
                            in1=linv, op=mybir.AluOpType.mult)
                    else:
                        o_tmp = npool.tile([64, TJ], BF16, name="o_tmp")
                        nc.gpsimd.tensor_tensor(
                            out=o_tmp, in0=o_stage[0:DH, :],
                            in1=linv, op=mybir.AluOpType.mult)
                        nc.sync.dma_start(
                            out=o_sb[hp][64:128, tsl], in_=o_tmp)

            # ---- interleaved emission --------------------------------
            # window 0 starts after only Q0/K0/V0; the rest of proj(0)
            # interleaves into window 0 ahead of proj(1), ordered so each
            # unit's Q/K land before that unit's first chunk.
            # Slot emission order is QK(si) first, AV trailing by 2 slots:
            # the PE never waits on EXP(si-1) before issuing QK(si), so the
            # exp stream decouples from the AV consume wait. Outproj is
            # deferred to window 3 (ACT-bound there: PE has ~30us of slack);
            # windows 0-2 carry only the projection chains they must.
            for th in proj_thunks(0, [(0, 0), (1, 0), (2, 0)]):
                th()
            rest0 = proj_thunks(0, [(2, 1), (2, 2), (2, 3), (0, 1), (1, 1),
                                    (0, 2), (1, 2), (0, 3), (1, 3)])
            from collections import deque
            for tj in range(NTJ):
                # x for slice tj+1 must land before the interleaved
                # proj(tj+1) chains read it (slices 0,1 pre-issued)
                if 2 <= tj + 1 < NTJ:
                    dma_x8(tj + 1)
                    dma_x(tj + 1)
                inserts = []
                if tj == 0:
                    inserts += rest0
                if tj + 1 < NTJ:
                    inserts += proj_thunks(tj + 1)
                if tj == NTJ - 1:
                    for t_ in range(NTJ - 1):
                        inserts += outproj_thunks(t_)
                nsi = 4 * tj + 4
                slots = []  # (kind, args) in emission order
                for hp in range(HG // 2):
                    for si in range(nsi):
                        slots.append(("c", hp, si))
                    slots.append(("n", hp, None))
                n_slots = len(slots)
                n_ins = len(inserts)
                acc = 0.0
                ii = 0
                pend = {}
                for j, s in enumerate(slots):
                    hp = s[1]
                    pq = pend.setdefault(hp, deque())
                    if s[0] == "c":
                        si = s[2]
                        pq.append((si, attn_qk(hp, tj, si)))
                        # QK leads; AVs drain two at a time at trail >= 2
                        if len(pq) >= 4:
                            for _ in range(2):
                                psi, pa = pq.popleft()
                                attn_av(hp, tj, psi, nsi, *pa)
                    else:
                        while pq:
                            psi, pa = pq.popleft()
                            attn_av(hp, tj, psi, nsi, *pa)
                        attn_norm(hp, tj)
                    acc += n_ins / n_slots
                    # fire inserts only at slot-pair boundaries so the
                    # [QK,QK][AV x4] groups stay adjacent on the PE queue
                    if j % 2 == 1 or s[0] == "n":
                        while ii < n_ins and acc >= 1.0:
                            inserts[ii]()
                            ii += 1
                            acc -= 1.0
                while ii < n_ins:
                    inserts[ii]()
                    ii += 1
            for th in outproj_thunks(NTJ - 1):
                th()

    nc.compile()
    return nc


def _get_nc():
    if "nc" not in _NC_CACHE:
        _NC_CACHE["nc"] = _build()
    return _NC_CACHE["nc"]


def _make_in_maps(x, Wq, Wk, Wv, Wp):
    bf = ml_dtypes.bfloat16
    tril_h = np.where(
        np.arange(128)[:, None] > np.arange(128)[None, :],
        np.float32(0.0), np.float32(1.0)).astype(np.float32)
    trilm_h = np.ascontiguousarray(
        np.broadcast_to(tril_h[:, None, :], (128, 2, 128))).astype(bf)
    f8 = ml_dtypes.float8_e4m3
    in_maps = []
    for core in range(8):
        b, g = core // 2, core % 2
        heads = range(g * HG, (g + 1) * HG)
        wq = np.concatenate([Wq[h] for h in heads], axis=1)
        wk = np.concatenate([Wk[h] for h in heads], axis=1)
        wv = np.concatenate([Wv[h] for h in heads], axis=1)
        xt_f = x[b].T.astype(np.float32)  # [C, T]
        xtt_h = np.ascontiguousarray(
            xt_f.reshape(CC, 128, NTJ, TJ).transpose(2, 1, 0, 3)).astype(bf)
        xtt8_h = np.ascontiguousarray(
            (xt_f * XS).reshape(CC, 128, NTJ, TJ)
            .transpose(2, 1, 0, 3)).astype(f8)
        # w8[p, qk, mg, cp, j, m] = WS * w[(2cp+j)*128+p, mg*128+m]
        w8_h = np.empty((128, 2, 4, 4, 2, 128), np.float32)
        for qk, w_ in ((0, wq), (1, wk)):
            wr4 = (w_ * WS).reshape(4, 2, 128, 4, 128)  # [cp, j, p, mg, m]
            w8_h[:, qk] = wr4.transpose(2, 3, 0, 1, 4)  # [p, mg, cp, j, m]
        wr_h = np.empty((128, 4 * C), np.float32)
        for c in range(CC):
            wr_h[:, c * 512:(c + 1) * 512] = wv[c * 128:(c + 1) * 128]
        in_maps.append({
            "xtt": xtt_h,
            "xtt8": xtt8_h,
            "w8": w8_h.astype(f8),
            "wr": wr_h.astype(bf),
            "wp": np.ascontiguousarray(
                Wp[g * HG * DH:(g + 1) * HG * DH, :]).astype(bf),
            "trilm": trilm_h,
            "ones8": np.ones((128, 64), bf),
        })
    return in_maps


_LAST_RESULTS = {}


def kernel(x, Wq, Wk, Wv, Wp, bp):
    x = np.asarray(x, np.float32)
    Wq = np.asarray(Wq, np.float32)
    Wk = np.asarray(Wk, np.float32)
    Wv = np.asarray(Wv, np.float32)
    Wp = np.asarray(Wp, np.float32)
    bp = np.asarray(bp, np.float32)

    nc = _get_nc()
    in_maps = _make_in_maps(x, Wq, Wk, Wv, Wp)
    res = bass_utils.run_bass_kernel_spmd(
        nc, in_maps, core_ids=list(range(8)), trace=TRACE)
    _LAST_RESULTS["res"] = res

    out = np.empty((B, T, C), np.float32)
    for b in range(B):
        out[b] = (res.results[2 * b]["out"].astype(np.float32)
                  + res.results[2 * b + 1]["out"].astype(np.float32) + bp)
    return out



# revision 43
# speedup vs baseline: 1.0284x; 1.0271x over previous
"""Multi-head causal attention (B=4, T=2048, C=1024, H=16, DH=64) on 8 trn2
NeuronCores.

Sharding: core = (batch, head-half): core 2*b+g computes heads g*8..g*8+8 of
batch b, including the partial output projection with the matching 512 rows
of Wp (tensor-parallel). Host-side unshard sums the two bf16 partials per
batch and adds bp.

Design (v2 ~240us, from ~289us baseline):
  - Q/K projections in fp8e4 DoubleRow (x at 16x, Wq/Wk at 64x, folded out
    via the copy scale and exp scale): same streaming rate as bf16 on this
    toolchain but half the instructions/LDWs. V/AV/outproj stay bf16 --
    fp8 there would put ~2.5% on the output values and fail the 2e-2 gate;
    the Q/K path is protected by the tiny 1/sqrt(C)=1/32 softmax scale.
  - QK row-tiled: each head's [64 x s] K slab sits at base partition 0/64;
    the two K=64 matmuls occupy disjoint PE row groups and dual-issue.
  - Causal mask: 0/1 tril multiply on DVE applied to the exp output's
    diagonal block (cheaper than the old PE mask-matmul accumulate).
  - ScalarE exp is the pacing engine late (~152us); emission order is the
    schedule. QK(si) is emitted BEFORE the trailing AVs (trail >= 2 slots,
    drained two at a time), so the exp feed never serializes behind the
    AV consume-wait -- this decoupling was worth ~20us.
  - Projection psum copy-outs ride ScalarE for windows 0-2 (ACT has slack
    there, DVE is the early bottleneck) and DVE for window 3's chains.
  - Window inserts: proj(tj+1) interleaves into window tj; ALL outproj for
    windows 0-2 is deferred into window 3 (ACT-bound there, PE has slack);
    outproj(3) + ScalarE casts form the tail.
  - Normalization: O^T/L with L from the V ones-column (psum row 64). o_ps
    frees via an immediate DVE copy (o_stage); the L DRAM-broadcast
    roundtrip + reciprocal run on DVE and the multiplies on GpSimd, all off
    the PE/ACT critical path.
  - Inputs arrive pre-tiled/packed from the host (x as [tj, p, c, 512]
    tiles bf16 + fp8, fp8 weights pre-packed for DoubleRow's [Ki,2,dim]
    AP) so every DMA is a single transfer; output is stored bf16.
"""
import numpy as np
import ml_dtypes

import concourse.mybir as mybir
import concourse.tile as tile
from concourse import bacc, bass_utils

F32 = mybir.dt.float32
BF16 = mybir.dt.bfloat16
FP8 = mybir.dt.float8e4
XS = 16.0             # fp8 scale for x
WS = 64.0             # fp8 scale for Wq/Wk
QKS = 1.0 / 64.0      # psum -> q_sb/k_sb copy scale (leaves q,k at 16x)

B, T, C, H, DH = 4, 2048, 1024, 16, 64
HG = H // 2          # heads per core (8)
CC = C // 128        # contraction chunks (8)
TJ = 512             # query chunk width
NTJ = T // TJ        # 4
SCALE = 1.0 / 32.0 / (16.0 * 16.0)   # 1/sqrt(C), q/k stored at 16x

TRACE = False
_NC_CACHE = {}


def _build():
    nc = bacc.Bacc(trn_type="TRN2", target_bir_lowering=False, debug=False)

    xtt = nc.dram_tensor("xtt", [NTJ, 128, CC, TJ], BF16,
                         kind="ExternalInput")
    xtt8 = nc.dram_tensor("xtt8", [NTJ, 128, CC, TJ], FP8,
                          kind="ExternalInput")
    # packed fp8 DoubleRow Q/K weights: [p, qk, mg, cpair, j, dhcol]
    w8 = nc.dram_tensor("w8", [128, 2, 4, 4, 2, 128], FP8,
                        kind="ExternalInput")
    wr = nc.dram_tensor("wr", [128, 4 * C], BF16, kind="ExternalInput")
    wp = nc.dram_tensor("wp", [HG * DH, C], BF16, kind="ExternalInput")
    trilm = nc.dram_tensor("trilm", [128, 2, 128], BF16, kind="ExternalInput")
    ones8 = nc.dram_tensor("ones8", [128, 64], BF16, kind="ExternalInput")
    out = nc.dram_tensor("out", [T, C], BF16, kind="ExternalOutput")

    l_dram = nc.dram_tensor("l_scratch", [HG * NTJ, TJ], F32)

    with tile.TileContext(nc) as tc:
        with (
            tc.tile_pool(name="persist", bufs=1) as persist,
            tc.tile_pool(name="ppool", bufs=6) as ppool,
            tc.tile_pool(name="npool", bufs=4) as npool,
            tc.tile_pool(name="outpool", bufs=4) as outpool,
            tc.tile_pool(name="pp", bufs=2, space="PSUM") as pp,
            tc.tile_pool(name="aps", bufs=2, space="PSUM") as aps,
        ):
            trilm_sb = persist.tile([128, 2, 128], BF16)
            nc.sync.dma_start(out=trilm_sb, in_=trilm.ap())
            ones_sb = persist.tile([128, 64], BF16)
            nc.sync.dma_start(out=ones_sb, in_=ones8.ap())

            # resident activations/weights, all bf16
            k_sb = [persist.tile([128, T], BF16, name=f"k_{mg}")
                    for mg in range(HG // 2)]
            q_sb = [persist.tile([128, T], BF16, name=f"q_{mg}")
                    for mg in range(HG // 2)]
            o_sb = [persist.tile([128, T], BF16, name=f"o_{mg}")
                    for mg in range(HG // 2)]
            v_sb = [persist.tile([128, HG, DH + 1], BF16, name=f"v_{si}")
                    for si in range(T // 128)]
            # bf16 weights: V columns only, 512 per c-chunk
            w_all = persist.tile([128, 4 * C], BF16)
            w8_sb = persist.tile([128, 2, 4, 4, 2, 128], FP8)
            xt_all = persist.tile([128, CC, T], BF16)
            xt8_all = persist.tile([128, CC, T], FP8)
            xt_sb = [xt_all[:, c, :] for c in range(CC)]

            def w_v(c):
                return w_all[:, c * 512:(c + 1) * 512]
            wp_sb = [persist.tile([128, C], BF16, name=f"wp_{hp}")
                     for hp in range(HG // 2)]

            def dma_x(tn):
                tsl = slice(tn * TJ, (tn + 1) * TJ)
                nc.sync.dma_start(out=xt_all[:, :, tsl],
                                  in_=xtt.ap()[tn])

            def dma_x8(tn):
                tsl = slice(tn * TJ, (tn + 1) * TJ)
                nc.sync.dma_start(out=xt8_all[:, :, tsl],
                                  in_=xtt8.ap()[tn])

            # first-needed first: fp8 q/k mg0 weights + x8(0) (q/k proj for
            # unit 0), then bf16 x(0) + v weights, then the rest.
            # First three loads ride three different engines' DGEs so their
            # ~0.6-1us trigger configs overlap instead of serializing on SP.
            # wp last: outproj doesn't run until window 3.
            nc.scalar.dma_start(out=w8_sb[:, :, 0:1], in_=w8.ap()[:, :, 0:1])
            dma_x8(0)
            dma_x(0)
            nc.sync.dma_start(out=w_all, in_=wr.ap())
            nc.sync.dma_start(out=w8_sb[:, :, 1:4], in_=w8.ap()[:, :, 1:4])
            dma_x8(1)
            dma_x(1)
            for hp in range(HG // 2):
                nc.sync.dma_start(out=wp_sb[hp],
                                  in_=wp.ap()[hp * 128:(hp + 1) * 128, :])

            o_ps = [aps.tile([DH + 1, TJ], F32, name=f"o_ps{i}", bufs=1)
                    for i in range(2)]

            # PE warm-up: ~3.4us of dead matmuls on the (tiny, first-loaded)
            # trilm tile while the big input DMAs land, so the HAM clock
            # gate opens (1.2->2.4GHz) before the first real proj chain.
            for _ in range(16):
                nc.tensor.matmul(
                    o_ps[0][:, 0:256], trilm_sb[:, 0, 0:65],
                    trilm_sb.rearrange("p a b -> p (a b)"),
                    start=True, stop=True)



            # ---- emission thunks -------------------------------------
            proj_state = {}

            def proj_half(tn, kind, mg, half):
                """One half of a projection psum chain.
                kind: 0=Q, 1=K (fp8 DoubleRow, 2 c-pairs per half),
                2=V (bf16, 4 c-chunks per half). half 1 copies out."""
                tsl = slice(tn * TJ, (tn + 1) * TJ)
                cs = range(4) if half == 0 else range(4, CC)
                key = (tn, kind, mg)
                if half == 0:
                    t_ = pp.tile([128, TJ], F32, name="prj", tag="pp")
                    proj_state[key] = t_
                else:
                    t_ = proj_state.pop(key)
                # psum->sbuf copy engine: ScalarE for windows 0-2's q/k
                # chains (ACT has slack there; DVE is the W0/W1 bottleneck),
                # DVE for V and for window 3's chains (ACT ~90% busy in W2).
                use_act = tn < 3 and kind != 2

                def ceng(dst, s_):
                    if use_act:
                        nc.scalar.copy(dst, s_)
                    else:
                        nc.vector.tensor_copy(dst, s_)
                if kind in (0, 1):
                    cps = range(2) if half == 0 else range(2, 4)
                    for cp in cps:
                        nc.tensor.matmul(
                            t_, w8_sb[:, kind, mg, cp],
                            xt8_all[:, 2 * cp:2 * cp + 2, tsl],
                            start=(cp == 0), stop=(cp == 3),
                            perf_mode=mybir.MatmulPerfMode.DoubleRow)
                    if half == 1:
                        dst = (q_sb if kind == 0 else k_sb)[mg][:, tsl]
                        if use_act:
                            nc.scalar.mul(dst, t_, QKS)
                        else:
                            nc.vector.tensor_scalar_mul(dst, t_, QKS)
                else:
                    si = 4 * tn + mg
                    ssl = slice(si * 128, (si + 1) * 128)
                    for c in cs:
                        nc.tensor.matmul(
                            t_, xt_sb[c][:, ssl], w_v(c),
                            start=(c == 0), stop=(c == CC - 1))
                    if half == 1:
                        ceng(
                            v_sb[si][:, :, 0:DH],
                            t_.rearrange("p (h d) -> p h d", h=HG))
                        nc.vector.tensor_copy(
                            out=v_sb[si][:, :, DH:DH + 1],
                            in_=ones_sb[:, 0:HG, None])

            def proj_thunks(tn, jobs=None):
                if jobs is None:
                    jobs = [(k, m) for m in range(4) for k in (0, 1, 2)]
                th = []
                for kind, mg in jobs:
                    for half in (0, 1):
                        th.append(lambda k=kind, m=mg, h=half:
                                  proj_half(tn, k, m, h))
                return th

            def outproj_tile(tj, k4, en):
                ti = 4 * tj + k4
                tsl = slice(ti * 128, (ti + 1) * 128)
                esl = slice(en * TJ, (en + 1) * TJ)
                op_ps = pp.tile([128, TJ], F32, name="op", tag="pp")
                for hp in range(HG // 2):
                    nc.tensor.matmul(
                        op_ps, o_sb[hp][:, tsl], wp_sb[hp][:, esl],
                        start=(hp == 0), stop=(hp == HG // 2 - 1))
                ob = outpool.tile([128, TJ], BF16, name="ob")
                if tj == NTJ - 1:
                    # tail: ScalarE is idle after the last exp
                    nc.scalar.copy(ob, op_ps)
                else:
                    nc.vector.tensor_copy(ob, op_ps)
                nc.sync.dma_start(out=out.ap()[tsl, esl], in_=ob)

            def outproj_thunks(tj):
                return [lambda k=k4, e=en: outproj_tile(tj, k, e)
                        for k4 in range(4) for en in range(2)]

            def attn_qk(hp, tj, si):
                r = si - 4 * tj
                toff = 0 if r < 0 else 128 * r
                ssl = slice(si * 128, (si + 1) * 128)
                qsl = slice(tj * TJ + toff, (tj + 1) * TJ)

                s_ps = aps.tile([128, 2, TJ], F32, name="s_ps", tag="s",
                                bufs=2)
                nc.tensor.matmul(
                    s_ps[:, 0, toff:TJ],
                    k_sb[hp][0:64, ssl], q_sb[hp][0:64, qsl],
                    start=True, stop=True)
                nc.tensor.matmul(
                    s_ps[:, 1, toff:TJ],
                    k_sb[hp][64:128, ssl], q_sb[hp][64:128, qsl],
                    start=True, stop=True)
                p_sb = ppool.tile([128, 2, TJ], BF16, name="p_sb", tag="p")
                nc.scalar.activation(
                    p_sb[:, :, toff:TJ], s_ps[:, :, toff:TJ],
                    mybir.ActivationFunctionType.Exp, scale=SCALE)
                if r >= 0:
                    # diag block: zero the above-diagonal entries of p with a
                    # 0/1 tril multiply on DVE (cheaper than PE mask matmuls)
                    nc.vector.tensor_tensor(
                        out=p_sb[:, :, toff:toff + 128],
                        in0=p_sb[:, :, toff:toff + 128],
                        in1=trilm_sb,
                        op=mybir.AluOpType.mult)
                return p_sb, toff

            def attn_av(hp, tj, si, nsi, p_sb, toff):
                h0, h1 = 2 * hp, 2 * hp + 1
                nc.tensor.matmul(
                    o_ps[0][:, toff:TJ], v_sb[si][:, h0, :],
                    p_sb[:, 0, toff:TJ],
                    start=(si == 0), stop=(si == nsi - 1))
                nc.tensor.matmul(
                    o_ps[1][:, toff:TJ], v_sb[si][:, h1, :],
                    p_sb[:, 1, toff:TJ],
                    start=(si == 0), stop=(si == nsi - 1))

            def attn_norm_last(hp, tj):
                # final unit: nothing reuses o_ps afterwards, so normalize
                # straight out of PSUM with L broadcast by a K=1 matmul
                # (ones stationary) instead of the DRAM roundtrip.
                tsl = slice(tj * TJ, (tj + 1) * TJ)
                for idx in range(2):
                    l_bf = npool.tile([DH + 1, TJ], BF16, name="l_bf")
                    nc.vector.tensor_copy(
                        l_bf[DH:DH + 1, :], o_ps[idx][DH:DH + 1, :])
                    lb_ps = pp.tile([128, TJ], F32, name="lbps", tag="pp")
                    nc.tensor.matmul(
                        lb_ps[0:64, :], ones_sb[DH:DH + 1, :],
                        l_bf[DH:DH + 1, :], start=True, stop=True)
                    linv = npool.tile([64, TJ], F32, name="linv")
                    nc.vector.reciprocal_approx_fast(linv, lb_ps[0:64, :])
                    if idx == 0:
                        nc.vector.tensor_tensor(
                            out=o_sb[hp][0:64, tsl], in0=o_ps[0][0:DH, :],
                            in1=linv, op=mybir.AluOpType.mult)
                    else:
                        o_tmp = npool.tile([64, TJ], BF16, name="o_tmp")
                        nc.vector.tensor_tensor(
                            out=o_tmp, in0=o_ps[1][0:DH, :],
                            in1=linv, op=mybir.AluOpType.mult)
                        nc.sync.dma_start(
                            out=o_sb[hp][64:128, tsl], in_=o_tmp)

            def attn_norm(hp, tj):
                if hp == HG // 2 - 1 and tj == NTJ - 1:
                    return attn_norm_last(hp, tj)
                # free o_ps fast (DVE copy), then normalize off-PE:
                # L row 64 -> DRAM -> broadcast to 64 partitions,
                # reciprocal, multiply; h1 lands via SBUF->SBUF DMA.
                tsl = slice(tj * TJ, (tj + 1) * TJ)
                for idx in range(2):
                    lrow = (hp * 2 + idx) * NTJ + tj
                    o_stage = npool.tile([DH + 1, TJ], F32, name="o_stage")
                    nc.vector.tensor_copy(o_stage, o_ps[idx])
                    nc.sync.dma_start(
                        out=l_dram.ap()[lrow:lrow + 1, :],
                        in_=o_stage[DH:DH + 1, :])
                    lb = npool.tile([64, TJ], F32, name="lb")
                    nc.sync.dma_start(
                        out=lb,
                        in_=l_dram.ap()[lrow:lrow + 1, :]
                        .to_broadcast((64, TJ)))
                    linv = npool.tile([64, TJ], F32, name="linv")
                    nc.vector.reciprocal_approx_fast(linv, lb)
                    if idx == 0:
                        nc.gpsimd.tensor_tensor(
                            out=o_sb[hp][0:64, tsl], in0=o_stage[0:DH, :],
                            in1=linv, op=mybir.AluOpType.mult)
                    else:
                        o_tmp = npool.tile([64, TJ], BF16, name="o_tmp")
                        nc.gpsimd.tensor_tensor(
                            out=o_tmp, in0=o_stage[0:DH, :],
                            in1=linv, op=mybir.AluOpType.mult)
                        nc.sync.dma_start(
                            out=o_sb[hp][64:128, tsl], in_=o_tmp)

            # ---- interleaved emission --------------------------------
            # window 0 starts after only Q0/K0/V0; the rest of proj(0)
            # interleaves into window 0 ahead of proj(1), ordered so each
            # unit's Q/K land before that unit's first chunk.
            # Slot emission order is QK(si) first, AV trailing by 2 slots:
            # the PE never waits on EXP(si-1) before issuing QK(si), so the
            # exp stream decouples from the AV consume wait. Outproj is
            # deferred to window 3 (ACT-bound there: PE has ~30us of slack);
            # windows 0-2 carry only the projection chains they must.
            for th in proj_thunks(0, [(0, 0), (1, 0), (2, 0)]):
                th()
            rest0 = proj_thunks(0, [(2, 1), (2, 2), (2, 3), (0, 1), (1, 1),
                                    (0, 2), (1, 2), (0, 3), (1, 3)])
            from collections import deque
            for tj in range(NTJ):
                # x for slice tj+1 must land before the interleaved
                # proj(tj+1) chains read it (slices 0,1 pre-issued)
                if 2 <= tj + 1 < NTJ:
                    dma_x8(tj + 1)
                    dma_x(tj + 1)
                inserts = []
                if tj == 0:
                    inserts += rest0
                if tj + 1 < NTJ:
                    inserts += proj_thunks(tj + 1)
                if tj == NTJ - 1:
                    for t_ in range(NTJ - 1):
                        inserts += outproj_thunks(t_)
                nsi = 4 * tj + 4
                slots = []  # (kind, args) in emission order
                for hp in range(HG // 2):
                    for si in range(nsi):
                        slots.append(("c", hp, si))
                    slots.append(("n", hp, None))
                n_slots = len(slots)
                n_ins = len(inserts)
                acc = 0.0
                ii = 0
                pend = {}
                for j, s in enumerate(slots):
                    hp = s[1]
                    pq = pend.setdefault(hp, deque())
                    if s[0] == "c":
                        si = s[2]
                        pq.append((si, attn_qk(hp, tj, si)))
                        # QK leads; AVs drain two at a time at trail >= 2
                        if len(pq) >= 4:
                            for _ in range(2):
                                psi, pa = pq.popleft()
                                attn_av(hp, tj, psi, nsi, *pa)
                    else:
                        while pq:
                            psi, pa = pq.popleft()
                            attn_av(hp, tj, psi, nsi, *pa)
                        attn_norm(hp, tj)
                    acc += n_ins / n_slots
                    # fire inserts only at slot-pair boundaries so the
                    # [QK,QK][AV x4] groups stay adjacent on the PE queue
                    if j % 2 == 1 or s[0] == "n":
                        while ii < n_ins and acc >= 1.0:
                            inserts[ii]()
                            ii += 1
                            acc -= 1.0
                while ii < n_ins:
                    inserts[ii]()
                    ii += 1
            for th in outproj_thunks(NTJ - 1):
                th()

    nc.compile()
    return nc


def _get_nc():
    if "nc" not in _NC_CACHE:
        _NC_CACHE["nc"] = _build()
    return _NC_CACHE["nc"]


def _make_in_maps(x, Wq, Wk, Wv, Wp):
    bf = ml_dtypes.bfloat16
    tril_h = np.where(
        np.arange(128)[:, None] > np.arange(128)[None, :],
        np.float32(0.0), np.float32(1.0)).astype(np.float32)
    trilm_h = np.ascontiguousarray(
        np.broadcast_to(tril_h[:, None, :], (128, 2, 128))).astype(bf)
    f8 = ml_dtypes.float8_e4m3
    in_maps = []
    for core in range(8):
        b, g = core // 2, core % 2
        heads = range(g * HG, (g + 1) * HG)
        wq = np.concatenate([Wq[h] for h in heads], axis=1)
        wk = np.concatenate([Wk[h] for h in heads], axis=1)
        wv = np.concatenate([Wv[h] for h in heads], axis=1)
        xt_f = x[b].T.astype(np.float32)  # [C, T]
        xtt_h = np.ascontiguousarray(
            xt_f.reshape(CC, 128, NTJ, TJ).transpose(2, 1, 0, 3)).astype(bf)
        xtt8_h = np.ascontiguousarray(
            (xt_f * XS).reshape(CC, 128, NTJ, TJ)
            .transpose(2, 1, 0, 3)).astype(f8)
        # w8[p, qk, mg, cp, j, m] = WS * w[(2cp+j)*128+p, mg*128+m]
        w8_h = np.empty((128, 2, 4, 4, 2, 128), np.float32)
        for qk, w_ in ((0, wq), (1, wk)):
            wr4 = (w_ * WS).reshape(4, 2, 128, 4, 128)  # [cp, j, p, mg, m]
            w8_h[:, qk] = wr4.transpose(2, 3, 0, 1, 4)  # [p, mg, cp, j, m]
        wr_h = np.empty((128, 4 * C), np.float32)
        for c in range(CC):
            wr_h[:, c * 512:(c + 1) * 512] = wv[c * 128:(c + 1) * 128]
        in_maps.append({
            "xtt": xtt_h,
            "xtt8": xtt8_h,
            "w8": w8_h.astype(f8),
            "wr": wr_h.astype(bf),
            "wp": np.ascontiguousarray(
                Wp[g * HG * DH:(g + 1) * HG * DH, :]).astype(bf),
            "trilm": trilm_h,
            "ones8": np.ones((128, 64), bf),
        })
    return in_maps


_LAST_RESULTS = {}


def kernel(x, Wq, Wk, Wv, Wp, bp):
    x = np.asarray(x, np.float32)
    Wq = np.asarray(Wq, np.float32)
    Wk = np.asarray(Wk, np.float32)
    Wv = np.asarray(Wv, np.float32)
    Wp = np.asarray(Wp, np.float32)
    bp = np.asarray(bp, np.float32)

    nc = _get_nc()
    in_maps = _make_in_maps(x, Wq, Wk, Wv, Wp)
    res = bass_utils.run_bass_kernel_spmd(
        nc, in_maps, core_ids=list(range(8)), trace=TRACE)
    _LAST_RESULTS["res"] = res

    out = np.empty((B, T, C), np.float32)
    for b in range(B):
        out[b] = (res.results[2 * b]["out"].astype(np.float32)
                  + res.results[2 * b + 1]["out"].astype(np.float32) + bp)
    return out

